# revision 17
# baseline (speedup 1.0000x reference)
"""Trainium2 Bass kernel for nn_Dist_Conv2D (Chebyshev-distance conv) — LSE/matmul.

out[b,o,h,w] = max_{c,kh,kw} |x_pad[b,c,h+kh,w+kw] - weights[o,c,kh,kw]| + bias[o]
x: [16,64,56,56] f32, weights: [128,64,3,3] f32, bias: [128,1,1] f32 -> [16,128,56,56].

Scheme: the Chebyshev max is computed on the TensorEngine via log-sum-exp:
  S[n,o] = sum_d e^{beta(x_nd - w_od) + C} + e^{beta(w_od - x_nd) + C}
         = (E+x @ E-w.T + E-x @ E+w.T)[n, o]
  out[n,o] = ln(S)/beta - C/beta - off + bias[o],  LSE bias ~= log(#near-ties)/beta.

Both sign-sides fuse into one K=128 matmul per conv tap: rhs partitions 0-63
hold e^{+beta*x+A} (channels of the padded image, channels-last), partitions
64-127 hold e^{-beta*x+A'}; lhsT rows stack e^{-beta*w+B} over e^{+beta*w+B'}.
9 taps x 7 position-tiles (N=512) accumulate in PSUM per image; then
ScalarE Ln, VectorE affine (+bias), DMA out. Exponent shifts A/B/A'/B' are
solved host-side from data extrema so every needed term stays inside
bf16/fp32 dynamic range (validated in sim: max abs err ~0.06 vs budget 0.21).

Data-parallel over batch: 2 images per core, 8 cores.
"""

import numpy as np
import ml_dtypes

import concourse.bacc as bacc
import concourse.mybir as mybir
from concourse.tile import TileContext
from concourse.bass_utils import run_bass_kernel_spmd

# ---------------------------------------------------------------------------
# Problem geometry (hardcoded).
# ---------------------------------------------------------------------------
B, CIN, H, W = 16, 64, 56, 56
COUT, K = 128, 3
PADL = 1                     # PADDING=2 split 1/1
HP, WP = H + 2, W + 2        # 58 x 58 padded image
NCORES = 8
B_PER = B // NCORES          # 2 images per core
IMG_COLS = HP * WP           # 3364 pixels per padded image (channels-last)
P = 128
NTILE = 512                  # positions per matmul / PSUM bank
TILES_PER_IMG = 7            # 7*512 = 3584 >= 56*58 = 3248 positions (halo trick)
POS_PER_IMG = TILES_PER_IMG * NTILE  # 3584
TAPS = [(kh, kw) for kh in range(K) for kw in range(K)]
OFFS = [kh * WP + kw for kh, kw in TAPS]   # column offsets per tap
NCOL = 3712                  # POS_PER_IMG-1 + max(OFFS) = 3583+118=3701 -> pad
BF16 = mybir.dt.bfloat16
F32 = mybir.dt.float32

_CACHE: dict = {}


def _build_program(loop_n=None, skip_mm=False, skip_post=False, batch_out=True,
                   tinner=True, even_offs=False, ksplit=False):
    key = ("lse", loop_n, skip_mm, skip_post, batch_out, tinner, even_offs, ksplit)
    if key in _CACHE:
        return _CACHE[key]
    offs = [o - (o % 2) for o in OFFS] if even_offs else OFFS
    nc = bacc.Bacc("TRN2", num_devices=NCORES)
    xs_ext = nc.declare_dram_parameter("xs", [B_PER * NCOL * CIN], BF16, isOutput=False)
    w_ext = nc.declare_dram_parameter("wl", [P, 9 * P], BF16, isOutput=False)
    c_ext = nc.declare_dram_parameter("cs", [P, 4], F32, isOutput=False)
    out_ext = nc.declare_dram_parameter(
        "out", [P, B_PER * POS_PER_IMG], BF16, isOutput=True
    )
    ap_cls = type(xs_ext[:].ap)

    with TileContext(nc) as tc:
        with tc.tile_pool(name="sbuf", bufs=1) as pool, \
             tc.tile_pool(name="psum", bufs=1, space="PSUM") as ppool:
            from contextlib import nullcontext

            loop_cm = tc.For_i(0, loop_n, 1) if loop_n else nullcontext()
            with loop_cm:
                # weights + constants (exp consts separate from post consts so
                # the next iteration's exp isn't WAR-serialized on this
                # iteration's ln/ts tail)
                csb = pool.tile([P, 4], F32, tag="cse", name="cse")
                nc.sync.dma_start(csb[:], c_ext[:])
                csp = pool.tile([P, 4], F32, tag="csp", name="csp")
                nc.sync.dma_start(csp[:], c_ext[:])
                w_sb = pool.tile([P, 9 * P], BF16)
                nc.sync.dma_start(w_sb[:], w_ext[:])
                w_img = [w_sb, w_sb]

                xr = []
                vb = []
                for b in range(B_PER):
                    xb = pool.tile([P, NCOL], BF16, tag=f"xr{b}")
                    # duplicate the channels-first image into both partition halves
                    for half in range(2):
                        src = xs_ext[:].copy()
                        src.offset = b * NCOL * CIN
                        src.ap = ap_cls([[NCOL, CIN], [1, NCOL]])
                        nc.sync.dma_start(xb[half * CIN : (half + 1) * CIN, :], src)
                    xr.append(xb)
                # exp for both halves in one ACT op (per-partition scale/bias),
                # chunked along columns so tile-0 matmuls start early
                EXPCH = (2176, NCOL - 2176)
                for b in range(B_PER):
                    v = pool.tile([P, NCOL], BF16, tag=f"vb{b}")
                    c0 = 0
                    for ch in EXPCH:
                        nc.scalar.activation(
                            v[:, c0 : c0 + ch], xr[b][:, c0 : c0 + ch],
                            mybir.ActivationFunctionType.Exp,
                            bias=csb[:, 1:2], scale=csb[:, 0:1],
                        )
                        c0 += ch
                    vb.append(v)

                for b in range(B_PER):
                    psums = [] if ksplit else [
                        ppool.tile([P, NTILE], F32, tag=f"ps{i}", name=f"ps{i}_{b}")
                        for i in range(TILES_PER_IMG)
                    ]
                    ob = (
                        pool.tile([P, POS_PER_IMG], BF16, tag=f"ob{b}", name=f"ob{b}")
                        if batch_out and not skip_post
                        else None
                    )

                    def post(i):
                        lnt = pool.tile([P, NTILE], F32, tag=f"ln{i % 2}",
                                        name=f"ln{i}_{b}")
                        nc.scalar.activation(
                            lnt[:], psums[i][:], mybir.ActivationFunctionType.Ln
                        )
                        if batch_out:
                            odst = ob[:, i * NTILE : (i + 1) * NTILE]
                        else:
                            osb = pool.tile([P, NTILE], BF16, tag=f"os{i % 2}_{b}",
                                            name=f"os{i}_{b}")
                            odst = osb[:]
                        nc.vector.tensor_scalar(
                            odst, lnt[:],
                            csp[:, 3:4], csp[:, 2:3],
                            mybir.AluOpType.mult, mybir.AluOpType.add,
                        )
                        if not batch_out:
                            dst = out_ext[:].copy()
                            dst.offset = b * POS_PER_IMG + i * NTILE
                            dst.ap = ap_cls([[B_PER * POS_PER_IMG, P], [1, NTILE]])
                            nc.sync.dma_start(dst, osb[:])

                    if not skip_mm:
                        if ksplit:
                            for i in range(TILES_PER_IMG):
                                psA = ppool.tile([P, NTILE], F32, tag=f"pA{i % 2}",
                                                 name=f"pA{i}_{b}")
                                psB = ppool.tile([P, NTILE], F32, tag=f"pB{i % 2}",
                                                 name=f"pB{i}_{b}")
                                for t in range(9):
                                    col = i * NTILE + offs[t]
                                    nc.tensor.matmul(
                                        psA[:],
                                        w_img[b][0:CIN, t * P : (t + 1) * P],
                                        vb[b][0:CIN, col : col + NTILE],
                                        start=(t == 0),
                                        stop=(t == 8),
                                    )
                                    nc.tensor.matmul(
                                        psB[:],
                                        w_img[b][CIN:P, t * P : (t + 1) * P],
                                        vb[b][CIN:P, col : col + NTILE],
                                        start=(t == 0),
                                        stop=(t == 8),
                                    )
                                sadd = pool.tile([P, NTILE], F32, tag=f"sa{i % 2}",
                                                 name=f"sa{i}_{b}")
                                nc.vector.tensor_tensor(
                                    sadd[:], psA[:], psB[:], mybir.AluOpType.add
                                )
                                if not skip_post:
                                    lnt = pool.tile([P, NTILE], F32, tag=f"ln{i % 2}",
                                                    name=f"kln{i}_{b}")
                                    nc.scalar.activation(
                                        lnt[:], sadd[:],
                                        mybir.ActivationFunctionType.Ln,
                                    )
                                    nc.vector.tensor_scalar(
                                        ob[:, i * NTILE : (i + 1) * NTILE], lnt[:],
                                        csp[:, 3:4], csp[:, 2:3],
                                        mybir.AluOpType.mult, mybir.AluOpType.add,
                                    )
                        elif tinner:
                            for i in range(TILES_PER_IMG):
                                for t in range(9):
                                    col = i * NTILE + offs[t]
                                    nc.tensor.matmul(
                                        psums[i][:],
                                        w_img[b][:, t * P : (t + 1) * P],
                                        vb[b][:, col : col + NTILE],
                                        start=(t == 0),
                                        stop=(t == 8),
                                    )
                                if not skip_post:
                                    post(i)
                        else:
                            for t in range(9):
                                lhsT = w_img[b][:, t * P : (t + 1) * P]
                                for i in range(TILES_PER_IMG):
                                    col = i * NTILE + offs[t]
                                    nc.tensor.matmul(
                                        psums[i][:],
                                        lhsT,
                                        vb[b][:, col : col + NTILE],
                                        start=(t == 0),
                                        stop=(t == 8),
                                    )
                            if not skip_post:
                                for i in range(TILES_PER_IMG):
                                    post(i)
                    elif not skip_post:
                        for i in range(TILES_PER_IMG):
                            post(i)
                    if batch_out and not skip_post:
                        dst = out_ext[:].copy()
                        dst.offset = b * POS_PER_IMG
                        dst.ap = ap_cls([[B_PER * POS_PER_IMG, P], [1, POS_PER_IMG]])
                        nc.sync.dma_start(dst, ob[:])

    nc.compile()
    _CACHE[key] = nc
    return nc


LN_WIN = 43.0  # device ACT Ln is accurate only for inputs in [e^-44.5, e^+44.5]


def _solve_consts(x, w, t_min=2.4, beta_cap=20.0):
    """Choose beta and exponent shifts so every term that can be the max
    stays in dynamic range AND ln(S) = beta*out + C lands inside the device
    Ln table's safe window (see sim_lse2.py / probe_ln.py)."""
    X0 = float(np.max(x)); X1 = float(-np.min(x))
    W0 = float(np.max(w)); W1 = float(-np.min(w))
    xc_max = x.max(axis=(0, 2, 3)); xc_min = x.min(axis=(0, 2, 3))
    wc_max = w.max(axis=(0, 2, 3)); wc_min = w.min(axis=(0, 2, 3))
    Tm = max(float(np.max(xc_max - wc_min)), float(np.max(wc_max - xc_min)))

    def window(beta):
        # C window: product survival / no-overflow / operand-cut harmlessness
        L = max(-80.0 - beta * t_min,
                beta * (X0 + W1 - 2 * t_min) - 170.0,
                beta * (X1 + W0 - 2 * t_min) - 170.0,
                -LN_WIN - beta * t_min)          # ln window low edge
        U = min(83.0 - beta * Tm,
                LN_WIN - beta * Tm)              # ln window high edge
        return L, U

    beta = beta_cap
    while beta > 6.0:
        L, U = window(beta)
        if U - L >= 2.0:
            break
        beta -= 0.25
    L, U = window(beta)
    C = (L + U) / 2.0
    A_lo = -85.0 + beta * (W1 - t_min); A_hi = 84.0 - beta * X0
    B_lo = -85.0 + beta * (X0 - t_min); B_hi = 84.0 - beta * W1
    A = (max(A_lo, C - B_hi) + min(A_hi, C - B_lo)) / 2.0
    Ap_lo = -85.0 + beta * (W0 - t_min); Ap_hi = 84.0 - beta * X1
    Bp_lo = -85.0 + beta * (X1 - t_min); Bp_hi = 84.0 - beta * W0
    A2 = (max(Ap_lo, C - Bp_hi) + min(Ap_hi, C - Bp_lo)) / 2.0
    return beta, C, A, C - A, A2, C - A2


def _prep_inputs(x, weights, bias):
    x = np.asarray(x, dtype=np.float32)
    weights = np.asarray(weights, dtype=np.float32)
    bias = np.asarray(bias, dtype=np.float32).reshape(COUT)

    beta, C, A, Bc, A2, B2 = _solve_consts(x, weights)

    # x: pad, channels-first [b, c, pixel], bf16, zero-padded to NCOL pixels
    xp = np.pad(x, ((0, 0), (0, 0), (PADL, PADL), (PADL, PADL)))
    xbuf = np.zeros((B, CIN, NCOL), dtype=ml_dtypes.bfloat16)
    xbuf[:, :, :IMG_COLS] = xp.reshape(B, CIN, IMG_COLS).astype(ml_dtypes.bfloat16)

    # weights: [128 rows, 9*128]; row r<64: e^{-b w + B} chan r of tap t,
    # row r>=64: e^{+b w + B'} chan r-64.  col = t*128 + o.
    wd = weights.astype(np.float64)
    wl = np.empty((P, 9 * P), dtype=np.float64)
    for t, (kh, kw) in enumerate(TAPS):
        wt = wd[:, :, kh, kw]  # [o, c]
        wl[:CIN, t * P : (t + 1) * P] = np.exp(-beta * wt.T + Bc)
        wl[CIN:, t * P : (t + 1) * P] = np.exp(beta * wt.T + B2)
    wl = wl.astype(ml_dtypes.bfloat16)

    OFF_CENTER = 0.88 / beta
    consts = np.empty((P, 4), dtype=np.float32)
    consts[:CIN, 0] = beta
    consts[CIN:, 0] = -beta
    consts[:CIN, 1] = A
    consts[CIN:, 1] = A2
    consts[:, 2] = bias - C / beta - OFF_CENTER
    consts[:, 3] = 1.0 / beta

    in_maps = []
    for core in range(NCORES):
        xs = np.ascontiguousarray(
            xbuf[core * B_PER : (core + 1) * B_PER].reshape(-1)
        )
        in_maps.append({"xs": xs, "wl": wl, "cs": consts})
    return in_maps


def _unshard(results):
    outs = []
    for core in range(NCORES):
        r = results[core]["out"]  # [128, B_PER*POS_PER_IMG] bf16
        r = np.asarray(r, dtype=np.float32).reshape(P, B_PER, POS_PER_IMG)
        r = r[:, :, : H * WP].reshape(P, B_PER, H, WP)[:, :, :, :W]
        outs.append(r.transpose(1, 0, 2, 3))  # [B_PER, COUT, H, W]
    return np.concatenate(outs, axis=0)


def kernel(x, weights, bias):
    nc = _build_program()
    in_maps = _prep_inputs(x, weights, bias)
    res = run_bass_kernel_spmd(nc, in_maps, core_ids=list(range(NCORES)))
    return _unshard(res.results)


# revision 18
# speedup vs baseline: 1.1832x; 1.1832x over previous
"""Trainium2 Bass kernel for nn_Dist_Conv2D (Chebyshev-distance conv) — LSE/matmul.

out[b,o,h,w] = max_{c,kh,kw} |x_pad[b,c,h+kh,w+kw] - weights[o,c,kh,kw]| + bias[o]
x: [16,64,56,56] f32, weights: [128,64,3,3] f32, bias: [128,1,1] f32 -> [16,128,56,56].

Scheme: the Chebyshev max is computed on the TensorEngine via log-sum-exp:
  S[n,o] = sum_d e^{beta(x_nd - w_od) + C} + e^{beta(w_od - x_nd) + C}
         = (E+x @ E-w.T + E-x @ E+w.T)[n, o]
  out[n,o] = ln(S)/beta - C/beta - off + bias[o],  LSE bias ~= log(#near-ties)/beta.

Both sign-sides fuse into one K=128 matmul per conv tap: rhs partitions 0-63
hold e^{+beta*x+A} (channels of the padded image, channels-last), partitions
64-127 hold e^{-beta*x+A'}; lhsT rows stack e^{-beta*w+B} over e^{+beta*w+B'}.
9 taps x 7 position-tiles (N=512) accumulate in PSUM per image; then
ScalarE Ln, VectorE affine (+bias), DMA out. Exponent shifts A/B/A'/B' are
solved host-side from data extrema so every needed term stays inside
bf16/fp32 dynamic range (validated in sim: max abs err ~0.06 vs budget 0.21).

Data-parallel over batch: 2 images per core, 8 cores.
"""

import numpy as np
import ml_dtypes

import concourse.bacc as bacc
import concourse.mybir as mybir
from concourse.tile import TileContext
from concourse.bass_utils import run_bass_kernel_spmd

# ---------------------------------------------------------------------------
# Problem geometry (hardcoded).
# ---------------------------------------------------------------------------
B, CIN, H, W = 16, 64, 56, 56
COUT, K = 128, 3
PADL = 1                     # PADDING=2 split 1/1
HP, WP = H + 2, W + 2        # 58 x 58 padded image
NCORES = 8
B_PER = B // NCORES          # 2 images per core
IMG_COLS = HP * WP           # 3364 pixels per padded image (channels-last)
P = 128
NTILE = 512                  # positions per matmul / PSUM bank
TILES_PER_IMG = 7            # 7*512 = 3584 >= 56*58 = 3248 positions (halo trick)
POS_PER_IMG = TILES_PER_IMG * NTILE  # 3584
TAPS = [(kh, kw) for kh in range(K) for kw in range(K)]
OFFS = [kh * WP + kw for kh, kw in TAPS]   # column offsets per tap
NCOL = 3712                  # POS_PER_IMG-1 + max(OFFS) = 3583+118=3701 -> pad
BF16 = mybir.dt.bfloat16
F32 = mybir.dt.float32

_CACHE: dict = {}


def _build_program(loop_n=None, skip_mm=False, skip_post=False, batch_out=True,
                   tinner=True, even_offs=False, ksplit=False):
    key = ("lse", loop_n, skip_mm, skip_post, batch_out, tinner, even_offs, ksplit)
    if key in _CACHE:
        return _CACHE[key]
    offs = [o - (o % 2) for o in OFFS] if even_offs else OFFS
    nc = bacc.Bacc("TRN2", num_devices=NCORES)
    xs_ext = nc.declare_dram_parameter("xs", [B_PER * NCOL * CIN], BF16, isOutput=False)
    w_ext = nc.declare_dram_parameter("wl", [P, 9 * P], BF16, isOutput=False)
    c_ext = nc.declare_dram_parameter("cs", [P, 4], F32, isOutput=False)
    out_ext = nc.declare_dram_parameter(
        "out", [P, B_PER * POS_PER_IMG], BF16, isOutput=True
    )
    ap_cls = type(xs_ext[:].ap)

    with TileContext(nc) as tc:
        with tc.tile_pool(name="sbuf", bufs=1) as pool, \
             tc.tile_pool(name="psum", bufs=1, space="PSUM") as ppool:
            from contextlib import nullcontext

            loop_cm = tc.For_i(0, loop_n, 1) if loop_n else nullcontext()
            with loop_cm:
                # weights + constants (exp consts separate from post consts so
                # the next iteration's exp isn't WAR-serialized on this
                # iteration's ln/ts tail)
                csb = pool.tile([P, 4], F32, tag="cse", name="cse")
                nc.sync.dma_start(csb[:], c_ext[:])
                csp = pool.tile([P, 4], F32, tag="csp", name="csp")
                nc.sync.dma_start(csp[:], c_ext[:])
                w_sb = pool.tile([P, 9 * P], BF16)
                nc.sync.dma_start(w_sb[:], w_ext[:])
                w_img = [w_sb, w_sb]

                xr = []
                vb = []
                for b in range(B_PER):
                    xb = pool.tile([P, NCOL], BF16, tag=f"xr{b}")
                    # duplicate the channels-first image into both partition halves
                    for half in range(2):
                        src = xs_ext[:].copy()
                        src.offset = b * NCOL * CIN
                        src.ap = ap_cls([[NCOL, CIN], [1, NCOL]])
                        nc.sync.dma_start(xb[half * CIN : (half + 1) * CIN, :], src)
                    xr.append(xb)
                # exp for both halves in one ACT op (per-partition scale/bias),
                # chunked along columns so tile-0 matmuls start early
                EXPCH = (640, 1536, NCOL - 2176)
                for b in range(B_PER):
                    v = pool.tile([P, NCOL], BF16, tag=f"vb{b}")
                    c0 = 0
                    for ch in EXPCH:
                        nc.scalar.activation(
                            v[:, c0 : c0 + ch], xr[b][:, c0 : c0 + ch],
                            mybir.ActivationFunctionType.Exp,
                            bias=csb[:, 1:2], scale=csb[:, 0:1],
                        )
                        c0 += ch
                    vb.append(v)

                for b in range(B_PER):
                    psums = [] if ksplit else [
                        ppool.tile([P, NTILE], F32, tag=f"ps{i}", name=f"ps{i}_{b}")
                        for i in range(TILES_PER_IMG)
                    ]
                    ob = (
                        pool.tile([P, POS_PER_IMG], BF16, tag=f"ob{b}", name=f"ob{b}")
                        if batch_out and not skip_post
                        else None
                    )

                    def post(i):
                        lnt = pool.tile([P, NTILE], F32, tag=f"ln{i % 2}",
                                        name=f"ln{i}_{b}")
                        nc.scalar.activation(
                            lnt[:], psums[i][:], mybir.ActivationFunctionType.Ln
                        )
                        if batch_out:
                            odst = ob[:, i * NTILE : (i + 1) * NTILE]
                        else:
                            osb = pool.tile([P, NTILE], BF16, tag=f"os{i % 2}_{b}",
                                            name=f"os{i}_{b}")
                            odst = osb[:]
                        nc.vector.tensor_scalar(
                            odst, lnt[:],
                            csp[:, 3:4], csp[:, 2:3],
                            mybir.AluOpType.mult, mybir.AluOpType.add,
                        )
                        if not batch_out:
                            dst = out_ext[:].copy()
                            dst.offset = b * POS_PER_IMG + i * NTILE
                            dst.ap = ap_cls([[B_PER * POS_PER_IMG, P], [1, NTILE]])
                            nc.sync.dma_start(dst, osb[:])

                    if not skip_mm:
                        if ksplit:
                            for i in range(TILES_PER_IMG):
                                psA = ppool.tile([P, NTILE], F32, tag=f"pA{i % 2}",
                                                 name=f"pA{i}_{b}")
                                psB = ppool.tile([P, NTILE], F32, tag=f"pB{i % 2}",
                                                 name=f"pB{i}_{b}")
                                for t in range(9):
                                    col = i * NTILE + offs[t]
                                    nc.tensor.matmul(
                                        psA[:],
                                        w_img[b][0:CIN, t * P : (t + 1) * P],
                                        vb[b][0:CIN, col : col + NTILE],
                                        start=(t == 0),
                                        stop=(t == 8),
                                    )
                                    nc.tensor.matmul(
                                        psB[:],
                                        w_img[b][CIN:P, t * P : (t + 1) * P],
                                        vb[b][CIN:P, col : col + NTILE],
                                        start=(t == 0),
                                        stop=(t == 8),
                                    )
                                sadd = pool.tile([P, NTILE], F32, tag=f"sa{i % 2}",
                                                 name=f"sa{i}_{b}")
                                nc.vector.tensor_tensor(
                                    sadd[:], psA[:], psB[:], mybir.AluOpType.add
                                )
                                if not skip_post:
                                    lnt = pool.tile([P, NTILE], F32, tag=f"ln{i % 2}",
                                                    name=f"kln{i}_{b}")
                                    nc.scalar.activation(
                                        lnt[:], sadd[:],
                                        mybir.ActivationFunctionType.Ln,
                                    )
                                    nc.vector.tensor_scalar(
                                        ob[:, i * NTILE : (i + 1) * NTILE], lnt[:],
                                        csp[:, 3:4], csp[:, 2:3],
                                        mybir.AluOpType.mult, mybir.AluOpType.add,
                                    )
                        elif tinner:
                            for i in range(TILES_PER_IMG):
                                for t in range(9):
                                    col = i * NTILE + offs[t]
                                    nc.tensor.matmul(
                                        psums[i][:],
                                        w_img[b][:, t * P : (t + 1) * P],
                                        vb[b][:, col : col + NTILE],
                                        start=(t == 0),
                                        stop=(t == 8),
                                    )
                                if not skip_post:
                                    post(i)
                        else:
                            for t in range(9):
                                lhsT = w_img[b][:, t * P : (t + 1) * P]
                                for i in range(TILES_PER_IMG):
                                    col = i * NTILE + offs[t]
                                    nc.tensor.matmul(
                                        psums[i][:],
                                        lhsT,
                                        vb[b][:, col : col + NTILE],
                                        start=(t == 0),
                                        stop=(t == 8),
                                    )
                            if not skip_post:
                                for i in range(TILES_PER_IMG):
                                    post(i)
                    elif not skip_post:
                        for i in range(TILES_PER_IMG):
                            post(i)
                    if batch_out and not skip_post:
                        dst = out_ext[:].copy()
                        dst.offset = b * POS_PER_IMG
                        dst.ap = ap_cls([[B_PER * POS_PER_IMG, P], [1, POS_PER_IMG]])
                        nc.sync.dma_start(dst, ob[:])

    nc.compile()
    _CACHE[key] = nc
    return nc


LN_WIN = 43.0  # device ACT Ln is accurate only for inputs in [e^-44.5, e^+44.5]


def _solve_consts(x, w, t_min=2.4, beta_cap=20.0):
    """Choose beta and exponent shifts so every term that can be the max
    stays in dynamic range AND ln(S) = beta*out + C lands inside the device
    Ln table's safe window (see sim_lse2.py / probe_ln.py)."""
    X0 = float(np.max(x)); X1 = float(-np.min(x))
    W0 = float(np.max(w)); W1 = float(-np.min(w))
    xc_max = x.max(axis=(0, 2, 3)); xc_min = x.min(axis=(0, 2, 3))
    wc_max = w.max(axis=(0, 2, 3)); wc_min = w.min(axis=(0, 2, 3))
    Tm = max(float(np.max(xc_max - wc_min)), float(np.max(wc_max - xc_min)))

    def window(beta):
        # C window: product survival / no-overflow / operand-cut harmlessness
        L = max(-80.0 - beta * t_min,
                beta * (X0 + W1 - 2 * t_min) - 170.0,
                beta * (X1 + W0 - 2 * t_min) - 170.0,
                -LN_WIN - beta * t_min)          # ln window low edge
        U = min(83.0 - beta * Tm,
                LN_WIN - beta * Tm)              # ln window high edge
        return L, U

    beta = beta_cap
    while beta > 6.0:
        L, U = window(beta)
        if U - L >= 2.0:
            break
        beta -= 0.25
    L, U = window(beta)
    C = (L + U) / 2.0
    A_lo = -85.0 + beta * (W1 - t_min); A_hi = 84.0 - beta * X0
    B_lo = -85.0 + beta * (X0 - t_min); B_hi = 84.0 - beta * W1
    A = (max(A_lo, C - B_hi) + min(A_hi, C - B_lo)) / 2.0
    Ap_lo = -85.0 + beta * (W0 - t_min); Ap_hi = 84.0 - beta * X1
    Bp_lo = -85.0 + beta * (X1 - t_min); Bp_hi = 84.0 - beta * W0
    A2 = (max(Ap_lo, C - Bp_hi) + min(Ap_hi, C - Bp_lo)) / 2.0
    return beta, C, A, C - A, A2, C - A2


def _prep_inputs(x, weights, bias):
    x = np.asarray(x, dtype=np.float32)
    weights = np.asarray(weights, dtype=np.float32)
    bias = np.asarray(bias, dtype=np.float32).reshape(COUT)

    beta, C, A, Bc, A2, B2 = _solve_consts(x, weights)

    # x: pad, channels-first [b, c, pixel], bf16, zero-padded to NCOL pixels
    xp = np.pad(x, ((0, 0), (0, 0), (PADL, PADL), (PADL, PADL)))
    xbuf = np.zeros((B, CIN, NCOL), dtype=ml_dtypes.bfloat16)
    xbuf[:, :, :IMG_COLS] = xp.reshape(B, CIN, IMG_COLS).astype(ml_dtypes.bfloat16)

    # weights: [128 rows, 9*128]; row r<64: e^{-b w + B} chan r of tap t,
    # row r>=64: e^{+b w + B'} chan r-64.  col = t*128 + o.
    wd = weights.astype(np.float64)
    wl = np.empty((P, 9 * P), dtype=np.float64)
    for t, (kh, kw) in enumerate(TAPS):
        wt = wd[:, :, kh, kw]  # [o, c]
        wl[:CIN, t * P : (t + 1) * P] = np.exp(-beta * wt.T + Bc)
        wl[CIN:, t * P : (t + 1) * P] = np.exp(beta * wt.T + B2)
    wl = wl.astype(ml_dtypes.bfloat16)

    OFF_CENTER = 0.88 / beta
    consts = np.empty((P, 4), dtype=np.float32)
    consts[:CIN, 0] = beta
    consts[CIN:, 0] = -beta
    consts[:CIN, 1] = A
    consts[CIN:, 1] = A2
    consts[:, 2] = bias - C / beta - OFF_CENTER
    consts[:, 3] = 1.0 / beta

    in_maps = []
    for core in range(NCORES):
        xs = np.ascontiguousarray(
            xbuf[core * B_PER : (core + 1) * B_PER].reshape(-1)
        )
        in_maps.append({"xs": xs, "wl": wl, "cs": consts})
    return in_maps


def _unshard(results):
    outs = []
    for core in range(NCORES):
        r = results[core]["out"]  # [128, B_PER*POS_PER_IMG] bf16
        r = np.asarray(r, dtype=np.float32).reshape(P, B_PER, POS_PER_IMG)
        r = r[:, :, : H * WP].reshape(P, B_PER, H, WP)[:, :, :, :W]
        outs.append(r.transpose(1, 0, 2, 3))  # [B_PER, COUT, H, W]
    return np.concatenate(outs, axis=0)


def kernel(x, weights, bias):
    nc = _build_program()
    in_maps = _prep_inputs(x, weights, bias)
    res = run_bass_kernel_spmd(nc, in_maps, core_ids=list(range(NCORES)))
    return _unshard(res.results)
